# revision 54
# baseline (speedup 1.0000x reference)
"""Trainium2 Bass kernel for CurvatureLoss3D.

Input phi [2,1,192,192,192] f32 -> scalar loss.

Sharding: 8 cores = (batch n in {0,1}) x (depth quarter). Each core gets an
input slab [50,192,192] (depth halo included) and computes per-(h,d)-row
partial sums of pen*mask and mask over its 48 output depth rows. Host trims
edge/overlap rows and finishes the scalar reduction.

On-chip layout: partitions = H, free = (shift s, D, W) where the DMA loads
three H-shifted replicas X3[p,s,d,w] = x(d, h0+p+s, w) via an overlapping
access pattern. A second replica Xc, shifted by +1 in w (SBUF->SBUF DMA),
makes every center-tap (w+1) read 4B-aligned so DVE runs in 2x bf16 mode.

Iteration packing: h-block 0 covers output h rows 0..125 with 128 partitions
(8 iterations over d). The remaining 64 h rows (126..189) are processed with
TWO depth-subblocks packed into the two 64-partition halves (4 iterations),
so every DVE/ACT op runs at full lane occupancy. The sign-sum band matmul
contracts over h (partitions); at each packed half's top two rows the
h-window crosses the half boundary, supplied instead from the s-replicas of
partition 63/127 via two extra accumulating matmuls (b12/b3) per (dd,dw).

Zero-crossing mask via sign-sum (27 neighbors all same sign <=> |sum| == 27).
Scaled-curvature algebra: with S2' = max(S2,1e-4)+4EPS, M3 = 0.5*S2'^1.5,
G = S2*trH - Ff: k1*M3 = G + sqrt(|G^2 - M3*G| + eps) and
1/(theta^2*M3^2) = (4/theta^2)*exp(-3 ln S2'), so the whole tail needs only
Ln/Exp/Square/Abs/Sign activations (ACT Reciprocal/Rsqrt are banned).
"""

import os
import sys

sys.path.insert(0, "/opt/trn_rl_repo")

import numpy as np

import bass_rust
import concourse.bass as bass
import concourse.tile as tile
from concourse import bacc
from concourse import mybir
from concourse.bass_utils import run_bass_kernel_spmd

F32 = mybir.dt.float32
BF16 = mybir.dt.bfloat16
ALU = mybir.AluOpType
ACTF = mybir.ActivationFunctionType
AX = mybir.AxisListType

EPS = 1e-8
THETA = 0.5 + 1e-8

N = 2
DVOL = 192
W = 192
DOUT = 190          # valid conv output extent per axis
D_IN = 50           # input slab depth rows per core
D_OUT_CORE = 48     # output depth rows computed per core
DB = 6              # output d rows per subblock
NSUB = D_OUT_CORE // DB
FD = DB * W         # pointwise free-dim extent
ROW = 3 * W         # one interleaved d-row in X3: shifts s=0,1,2 concatenated
X3W = (DB + 2) * ROW  # data cols in X3
X3PAD = X3W + 2     # +2 pad cols so trailing w+2 reads stay in-bounds
U3E = DB * ROW + 2  # U extent incl. w+1 read at s=2
DB2 = DB + 2        # sign path needs DB+2 d-rows
NPACK = NSUB // 2   # packed iterations covering h rows 126..189
NCOL = D_OUT_CORE + NPACK * DB  # accP/accC column count (48 + 24)

# per-core input-slab depth starts; output rows covered = d0..d0+47
CORE_D0 = [0, 48, 96, 142]

_last_results = None  # test harness reads exec time from here


def xo(s, d, w):
    return d * ROW + s * W + w


def _emit(tc, x, band, outp, outc, dbg=None):
    nc = tc.nc
    import contextlib
    import math

    with contextlib.ExitStack() as ctx:
        xpool = ctx.enter_context(tc.tile_pool(name="xin", bufs=2))
        cpool = ctx.enter_context(tc.tile_pool(name="xc", bufs=2))
        mpool = ctx.enter_context(tc.tile_pool(name="main", bufs=2))
        apool = ctx.enter_context(tc.tile_pool(name="acc", bufs=1))
        ppool = ctx.enter_context(tc.tile_pool(name="ps", bufs=2, space="PSUM"))

        # merged accumulator: per iteration, cols [2c, 2c+6) hold mask counts
        # and [2c+6, 2c+12) hold pen sums (one fused tensor_reduce per iter)
        acc = apool.tile([128, 2 * NCOL], F32, tag="acc", name="acc")
        nc.vector.memset(acc[:], 0.0)
        bandt = apool.tile([128, 4 * 128], BF16, tag="band", name="bandt")
        nc.sync.dma_start(bandt[:, :], band)

        # bias constants for ACT (only 0.0/1.0 are pre-registered)
        bias_tiles = {}
        for i, bval in enumerate(
            (2e-4, EPS, -1.5 * math.log(2.0), math.log(8.0 / THETA**2))
        ):
            bt = apool.tile([128, 1], F32, tag=f"bias{i}", name=f"bias{i}")
            nc.gpsimd.memset(bt[:], bval)
            bias_tiles[bval] = bt

        def BIAS(v):
            return bias_tiles[v][:, :]

        def T(tag, fd=FD, dt=BF16):
            return mpool.tile([128, fd], dt, tag=tag, name=tag)

        TT = nc.vector.tensor_tensor
        TS = nc.vector.tensor_scalar
        TSS = nc.vector.tensor_single_scalar
        ACT = nc.scalar.activation

        # iteration schedule: (packed, h0, din_a, din_b, col). Packed
        # iterations carry ~2.5x the PE work of unpacked ones, so interleave
        # them among the unpacked iterations to absorb the PE debt.
        unpacked = [(False, 0, DB * j, 0, DB * j) for j in range(NSUB)]
        packed_l = [
            (True, 126, 2 * DB * jj, 2 * DB * jj + DB, D_OUT_CORE + DB * jj)
            for jj in range(NPACK)
        ]
        iters = []
        for k in range(NSUB):
            iters.append(unpacked[k])
            if k % 2 == 0 and packed_l:
                iters.append(packed_l.pop(0))
        iters.extend(packed_l)

        for it, (packed, h0, din_a, din_b, col) in enumerate(iters):
            def DUMP(nm, t):
                if dbg is not None and it == 0 and nm in dbg:
                    nc.gpsimd.dma_start(dbg[nm], t)

            Xb = xpool.tile([128, X3PAD], BF16, tag="Xb", name="Xb")
            nc.gpsimd.memset(Xb[:, X3W:X3PAD], 1.0)
            if packed:
                for half, din0 in ((0, din_a), (1, din_b)):
                    src = x.copy()
                    src.offset = din0 * DVOL * W + h0 * W
                    src.ap = bass_rust.VecI64Pair(
                        [[W, 64], [DVOL * W, DB + 2], [1, ROW]]
                    )
                    nc.sync.dma_start(
                        Xb[64 * half : 64 * half + 64, 0:X3W], src
                    )
            else:
                src = x.copy()
                src.offset = din_a * DVOL * W + h0 * W
                src.ap = bass_rust.VecI64Pair(
                    [[W, 128], [DVOL * W, DB + 2], [1, ROW]]
                )
                nc.sync.dma_start(Xb[0:128, 0:X3W], src)
            # w+1-shifted replica loaded straight from DRAM (same linear
            # stream offset by one element; host pads the slab by 2) so it
            # runs in parallel with the Xb DMA instead of serializing on it
            Xc = cpool.tile([128, X3PAD], BF16, tag="Xc", name="Xc")
            nc.gpsimd.memset(Xc[:, X3W:X3PAD], 1.0)
            if packed:
                for half, din0 in ((0, din_a), (1, din_b)):
                    src = x.copy()
                    src.offset = din0 * DVOL * W + h0 * W + 1
                    src.ap = bass_rust.VecI64Pair(
                        [[W, 64], [DVOL * W, DB + 2], [1, ROW]]
                    )
                    nc.sync.dma_start(
                        Xc[64 * half : 64 * half + 64, 0:X3W], src
                    )
            else:
                src = x.copy()
                src.offset = din_a * DVOL * W + h0 * W + 1
                src.ap = bass_rust.VecI64Pair(
                    [[W, 128], [DVOL * W, DB + 2], [1, ROW]]
                )
                nc.sync.dma_start(Xc[0:128, 0:X3W], src)

            def xb(s, d, w, n=W):
                return _view2(Xb, xo(s, d, w), ROW, DB, n)

            def xc(s, d, w, n=W):
                return _view2(Xc, xo(s, d, w), ROW, DB, n)

            # ---- sign field early (feeds PE) ----
            # full X3 width: the packed-path fix matmuls need s=1,2 replicas
            sgf = T("sgF", X3PAD)
            ACT(sgf[:, :], Xb[:, :], ACTF.Sign)

            # ---- stencil fields (bf16, all reads 4B-aligned) ----
            U3 = T("U3", U3E)  # d-derivative of Xc, all 3 shifts
            TT(U3[:, 0:U3E], Xc[:, 2 * ROW : 2 * ROW + U3E],
               Xc[:, 0:U3E], ALU.subtract)

            def uoc(s, d, w, n=W):
                return _view2(U3, xo(s, d, w), ROW, DB, n)

            Vr = T("Vr", DB * 194)  # 2gy on 194-wide rows (w0 base)
            TT(_view2(Vr, 0, 194, DB, 194),
               _view2(Xb, xo(2, 1, 0), ROW, DB, 194),
               _view2(Xb, xo(0, 1, 0), ROW, DB, 194), ALU.subtract)

            def vv(w, n=W):
                return _view2(Vr, w, 194, DB, n)

            Vc = T("Vc")  # 2gy centered (aligned)
            TT(dnv(Vc), xc(2, 1, 0), xc(0, 1, 0), ALU.subtract)

            t1 = T("t1")
            TT(dnv(t1), xc(1, 0, 0), xc(1, 2, 0), ALU.add)
            t2 = T("t2")
            TT(dnv(t2), xb(1, 1, 0), xb(1, 1, 2), ALU.add)
            t3 = T("t3")
            TT(dnv(t3), xc(0, 1, 0), xc(2, 1, 0), ALU.add)
            x2c = T("s4")  # 2*x(d+1,h+1,w+1)  (on ACT: frees DVE cycles)
            ACT(dnv(x2c), xc(1, 1, 0), ACTF.Copy, scale=2.0)
            A = T("A")  # hxx
            TT(A[:, :], t1[:, :], x2c[:, :], ALU.subtract)
            C0 = T("C0")  # hzz
            TT(C0[:, :], t2[:, :], x2c[:, :], ALU.subtract)
            B = T("B")  # hyy
            TT(B[:, :], t3[:, :], x2c[:, :], ALU.subtract)
            W1 = T("W1")  # 2gz
            TT(dnv(W1), xb(1, 1, 2), xb(1, 1, 0), ALU.subtract)
            P = T("P")  # 4hxy (aligned via U3c)
            TT(dnv(P), uoc(2, 0, 0), uoc(0, 0, 0), ALU.subtract)
            # 4hxz = u[w+2]-u[w]: odd-offset views of U3 (1x DVE mode but
            # one op instead of three)
            Q = T("Q")  # 4hxz
            TT(Q[:, :],
               _view2(U3, xo(1, 0, 0) + 1, ROW, DB, W),
               _view2(U3, xo(1, 0, 0) - 1, ROW, DB, W), ALU.subtract)
            R = T("R")  # 4hyz
            TT(dnv(R), vv(2), vv(0), ALU.subtract)

            # ---- squares scaled by 2 (ACT, one table): U2 = 2u^2 etc., so
            # G = 2*(S2*trH - F) falls out of the X-products with no 0.5 op;
            # downstream constants absorb the factor 2 ----
            SQ2 = math.sqrt(2.0)
            U2 = T("U2")
            ACT(dnv(U2), uoc(1, 0, 0), ACTF.Square, scale=SQ2)
            V2 = T("V2")
            ACT(V2[:, :], Vc[:, :], ACTF.Square, scale=SQ2)
            W2s = T("W2s")
            ACT(W2s[:, :], W1[:, :], ACTF.Square, scale=SQ2)

            # ---- S2 = 8|g|^2; scaled-curvature reciprocal cluster ----
            S2 = T("S2")
            TT(S2[:, :], U2[:, :], V2[:, :], ALU.add)
            TT(S2[:, :], S2[:, :], W2s[:, :], ALU.add)
            DUMP("S2", S2[:, :])
            # the 2e-4 bias keeps mag^3 >> EPS so the scaled algebra stays
            # exact; only near-zero-gradient voxels (a handful, ref pen ~1e5
            # of a ~3e9 total) are perturbed
            Ltile = T("cC", FD, F32)  # ln(2*S2old+2e-4)
            ACT(Ltile[:, :], S2[:, :], ACTF.Ln, bias=BIAS(2e-4))
            m3h = T("cA")  # 2*M3 = (S2old')^1.5 = exp(1.5L)/2^1.5
            ACT(m3h[:, :], Ltile[:, :], ACTF.Exp, scale=1.5,
                bias=BIAS(-1.5 * math.log(2.0)))
            rr = T("R3")  # 1/(theta*2*M3)^2 = (8/theta^2)*exp(-3L)
            ACT(rr[:, :], Ltile[:, :], ACTF.Exp, scale=-3.0,
                bias=BIAS(math.log(8.0 / THETA**2)))

            # Fc = uvP + uwQ + vwR = u*(vP + wQ) + (vw)*R
            vP = T("s0")
            TT(vP[:, :], Vc[:, :], P[:, :], ALU.mult)
            wQ = T("s1")
            TT(wQ[:, :], W1[:, :], Q[:, :], ALU.mult)
            TT(vP[:, :], vP[:, :], wQ[:, :], ALU.add)
            Fc = T("s2")
            TT(dnv(Fc), uoc(1, 0, 0), _view2(vP, 0, W, DB, W), ALU.mult)
            vw = T("s1")
            TT(vw[:, :], Vc[:, :], W1[:, :], ALU.mult)
            TT(vw[:, :], vw[:, :], R[:, :], ALU.mult)
            TT(Fc[:, :], Fc[:, :], vw[:, :], ALU.add)

            # G = 2*(S2*trH - F) = U2*(B+C) + V2*(A+C) + W2s*(A+B) - Fc
            BC = T("trH")
            TT(BC[:, :], B[:, :], C0[:, :], ALU.add)
            AC = T("t2")
            TT(AC[:, :], A[:, :], C0[:, :], ALU.add)
            AB = T("t3")
            TT(AB[:, :], A[:, :], B[:, :], ALU.add)
            G = T("s0")
            TT(G[:, :], U2[:, :], BC[:, :], ALU.mult)
            X2 = T("s1")
            TT(X2[:, :], V2[:, :], AC[:, :], ALU.mult)
            TT(G[:, :], G[:, :], X2[:, :], ALU.add)
            TT(X2[:, :], W2s[:, :], AB[:, :], ALU.mult)
            TT(G[:, :], G[:, :], X2[:, :], ALU.add)
            TT(G[:, :], G[:, :], Fc[:, :], ALU.subtract)

            # ---- curvature glue (bf16) ----
            # D = G^2 - M3*G factored as G*(G - M3): same two DVE ops but no
            # ACT Square on the serial dependency chain
            Gm = T("s3")
            TT(Gm[:, :], G[:, :], m3h[:, :], ALU.subtract)  # 2G - 2M3
            Dq = T("s1")
            TT(Dq[:, :], G[:, :], Gm[:, :], ALU.mult)  # 4*dq*M3^2
            ad = T("s3")
            ACT(ad[:, :], Dq[:, :], ACTF.Abs)
            lnD = T("cL", FD, F32)
            ACT(lnD[:, :], ad[:, :], ACTF.Ln, bias=BIAS(EPS))
            sqv = T("s1")
            ACT(sqv[:, :], lnD[:, :], ACTF.Exp, scale=0.5)
            num = T("s0")
            TT(num[:, :], G[:, :], sqv[:, :], ALU.add)  # k1*M3
            numsq = T("s2")
            ACT(numsq[:, :], num[:, :], ACTF.Square)
            k2 = T("s1")
            TT(k2[:, :], numsq[:, :], rr[:, :], ALU.mult)
            pen = T("s0")
            TS(pen[:, :], k2[:, :], -1.0, 0.0, ALU.add, ALU.max)
            DUMP("pen", pen[:, :])

            # ---- zero-crossing mask: 27-sum of signs via PE matmuls ----
            # h-window via 3-diag band over partitions; (d,w)-window via 9
            # shifted views. Packed iterations use the clipped band plus two
            # fix matmuls routing partition-63/127 s-replicas to the half's
            # top rows.
            sdp = ppool.tile([128, DB * 256], F32, tag="sdps", name="sdp")
            if packed:
                bmain = bandt[:, 128:256]
                b12 = bandt[:, 256:384]
                b3 = bandt[:, 384:512]
                for dd in range(3):
                    for dw in range(3):
                        last = dd == 2 and dw == 2
                        for dp in range(0, DB, 2):
                            nc.tensor.matmul(
                                _view2(sdp, dp * 256, 256, 2, W),
                                bmain,
                                _view2(sgf, xo(0, dd + dp, dw), ROW, 2, W),
                                start=(dd == 0 and dw == 0),
                                stop=False,
                            )
                        for dp in range(0, DB, 2):
                            nc.tensor.matmul(
                                _view2(sdp, dp * 256, 256, 2, W),
                                b12,
                                _view2(sgf, xo(1, dd + dp, dw), ROW, 2, W),
                                start=False,
                                stop=False,
                            )
                            nc.tensor.matmul(
                                _view2(sdp, dp * 256, 256, 2, W),
                                b3,
                                _view2(sgf, xo(2, dd + dp, dw), ROW, 2, W),
                                start=False,
                                stop=last,
                            )
            else:
                for dd in range(3):
                    for dw in range(3):
                        for dp in range(0, DB, 2):
                            nc.tensor.matmul(
                                _view2(sdp, dp * 256, 256, 2, W),
                                bandt[:, 0:128],
                                _view2(sgf, xo(0, dd + dp, dw), ROW, 2, W),
                                start=(dd == 0 and dw == 0),
                                stop=(dd == 2 and dw == 2),
                            )
            sd2 = T("t1")
            ACT(dnv(sd2), _view2(sdp, 0, 256, DB, W), ACTF.Square)
            # mask in rows 0..5 of mp, pen*mask in rows 6..11: one fused
            # per-d-row reduction covers both
            mp = T("mp", 2 * FD)
            mask = mp[:, 0:FD]
            TSS(mask, sd2[:, :], 728.5, ALU.is_lt)
            DUMP("mask", mask)
            TT(mp[:, FD : 2 * FD], pen[:, :], mask, ALU.mult)
            nc.vector.tensor_reduce(
                acc[:, 2 * col : 2 * col + 2 * DB],
                _view2(mp, 0, W, 2 * DB, DOUT), AX.X, ALU.add,
            )

        nc.sync.dma_start(outp, acc[:, :])


def dnv(t, w=0, n=W):
    """dense [d][192] tile view"""
    return _view2(t, w, W, DB, n)


def _install_ntff_hook_shim():
    """Recreate antenv.axon_hooks (absent in this image) so trace=True works."""
    import sys as _sys
    import types
    if "antenv.axon_hooks" in _sys.modules:
        return
    try:
        from trn_agent_boot.trn_boot import _ntff_profile_via_ctypes
        hook = _ntff_profile_via_ctypes("/opt/axon/libaxon_pjrt.so")
    except Exception as e:
        print("ntff shim failed:", e)
        hook = None
    mod = types.ModuleType("antenv.axon_hooks")
    _state = {"hook": hook}
    mod.get_axon_ntff_profile_hook = lambda: _state["hook"]
    mod.set_axon_ntff_profile_hook = lambda h: _state.update(hook=h)
    _sys.modules["antenv.axon_hooks"] = mod
    import antenv
    antenv.axon_hooks = mod


def _view2(t, off, dstep, dcnt, n):
    """AP view of tile t: all partitions, free dims [(dstep, dcnt), (1, n)] at off."""
    ap = t[:, 0:1].copy()
    base = ap.ap.to_list()
    pdim = base[0]
    ap.offset = ap.offset + off
    ap.ap = bass_rust.VecI64Pair([list(pdim), [dstep, dcnt], [1, n]])
    return ap


def _patch_act_tables():
    """Steer the act-table-load pass away from the exp-less `natural_log`
    set: with ln hidden from it, an ln-miss resolves to
    `natural_log_exp_and_others`, which covers every function this kernel
    uses (ln/exp/square/sign/abs) -> ~2 table loads total instead of 4 per
    iteration. Entry positions (= act_func_set ids) are unchanged, so the
    emitted ids stay consistent with the compiler's act_info.json.
    """
    if getattr(bacc, "_act_tables_patched", False):
        return
    orig = bacc.get_activation_tables

    def patched(arch):
        out = {}
        for name, s in orig(arch).items():
            if name == "natural_log":
                s = {f for f in s if f != ACTF.Ln}
            out[name] = s
        return out

    bacc.get_activation_tables = patched
    bacc._act_tables_patched = True


def _build_nc():
    _patch_act_tables()
    nc = bacc.Bacc("TRN2", target_bir_lowering=False, debug=False, num_devices=8)
    x = nc.dram_tensor("x", [D_IN * DVOL * W + 2], BF16, kind="ExternalInput")
    band = nc.dram_tensor("band", [128, 4 * 128], BF16, kind="ExternalInput")
    outp = nc.dram_tensor("outp", [128, 2 * NCOL], F32, kind="ExternalOutput")
    with tile.TileContext(nc) as tc:
        _emit(tc, x.ap(), band.ap(), outp.ap(), None)
    nc.finalize()
    return nc


def _band_matrices():
    import ml_dtypes
    bm = np.zeros((128, 4 * 128), dtype=ml_dtypes.bfloat16)
    # [0:128]   unclipped 3-diag: out[o] = sum_{k=o..o+2} in[k]
    # [128:256] clipped at the 64-partition half boundary
    # [256:384] b12: routes partition 63/127 (s=1 view) to outputs 62,63
    # [384:512] b3: routes partition 63/127 (s=2 view) to output 63
    for o in range(128):
        for k in range(o, min(o + 3, 128)):
            bm[k, o] = 1.0
            if k // 64 == o // 64:
                bm[k, 128 + o] = 1.0
    for base in (0, 64):
        bm[base + 63, 256 + base + 62] = 1.0
        bm[base + 63, 256 + base + 63] = 1.0
        bm[base + 63, 384 + base + 63] = 1.0
    return bm


def kernel(phi):
    global _last_results
    phi = np.asarray(phi)
    assert phi.shape == (N, 1, DVOL, DVOL, W), phi.shape
    nc = _build_nc()
    import ml_dtypes
    phib = phi.astype(ml_dtypes.bfloat16)
    bandm = _band_matrices()
    in_maps = []
    for c in range(8):
        n, q = divmod(c, 4)
        d0 = CORE_D0[q]
        # flat + 2-element pad: the w+1-shifted Xc DMA reads one element
        # past the slab end on the last packed iteration
        slab = np.concatenate(
            [phib[n, 0, d0 : d0 + D_IN].ravel(),
             np.zeros(2, dtype=phib.dtype)]
        )
        in_maps.append({"x": slab, "band": bandm})
    trace = bool(int(os.environ.get("KERNEL_TRACE", "0")))
    if trace:
        _install_ntff_hook_shim()
    res = run_bass_kernel_spmd(nc, in_maps, list(range(8)), trace=trace)
    _last_results = res
    tp = 0.0
    tcnt = 0.0
    for c in range(8):
        blk = res.results[c]["outp"].astype(np.float64)
        blk = blk.reshape(128, NSUB + NPACK, 2, DB)
        oc = blk[:, :, 0, :]  # mask counts
        op = blk[:, :, 1, :]  # pen sums
        dlo = 2 if (c % 4) == 3 else 0
        # h-block 0 (iters 0..7): iter j holds d = 6j+ci, h rows 0..125 valid
        ocu = oc[:, :NSUB].reshape(128, D_OUT_CORE)
        opu = op[:, :NSUB].reshape(128, D_OUT_CORE)
        tp += opu[:126, dlo:].sum()
        tcnt += ocu[:126, dlo:].sum()
        # packed h-block (h 126..189, iters 8..11): iter 8+jj holds
        # d = 12jj+ci for partitions 0..63 and d = 12jj+6+ci for 64..127
        opp = op[:, NSUB:]
        ocp = oc[:, NSUB:]
        tp += opp[64:].sum() + opp[:64, 1:].sum() + opp[:64, 0, dlo:].sum()
        tcnt += ocp[64:].sum() + ocp[:64, 1:].sum() + ocp[:64, 0, dlo:].sum()
    return np.float32(tp / (tcnt + EPS))


# revision 56
# speedup vs baseline: 1.0070x; 1.0070x over previous
"""Trainium2 Bass kernel for CurvatureLoss3D.

Input phi [2,1,192,192,192] f32 -> scalar loss.

Sharding: 8 cores = (batch n in {0,1}) x (depth quarter). Each core gets an
input slab [50,192,192] (depth halo included) and computes per-(h,d)-row
partial sums of pen*mask and mask over its 48 output depth rows. Host trims
edge/overlap rows and finishes the scalar reduction.

On-chip layout: partitions = H, free = (shift s, D, W) where the DMA loads
three H-shifted replicas X3[p,s,d,w] = x(d, h0+p+s, w) via an overlapping
access pattern. A second replica Xc, shifted by +1 in w (SBUF->SBUF DMA),
makes every center-tap (w+1) read 4B-aligned so DVE runs in 2x bf16 mode.

Iteration packing: h-block 0 covers output h rows 0..125 with 128 partitions
(8 iterations over d). The remaining 64 h rows (126..189) are processed with
TWO depth-subblocks packed into the two 64-partition halves (4 iterations),
so every DVE/ACT op runs at full lane occupancy. The sign-sum band matmul
contracts over h (partitions); at each packed half's top two rows the
h-window crosses the half boundary, supplied instead from the s-replicas of
partition 63/127 via two extra accumulating matmuls (b12/b3) per (dd,dw).

Zero-crossing mask via sign-sum (27 neighbors all same sign <=> |sum| == 27).
Scaled-curvature algebra: with S2' = max(S2,1e-4)+4EPS, M3 = 0.5*S2'^1.5,
G = S2*trH - Ff: k1*M3 = G + sqrt(|G^2 - M3*G| + eps) and
1/(theta^2*M3^2) = (4/theta^2)*exp(-3 ln S2'), so the whole tail needs only
Ln/Exp/Square/Abs/Sign activations (ACT Reciprocal/Rsqrt are banned).
"""

import os
import sys

sys.path.insert(0, "/opt/trn_rl_repo")

import numpy as np

import bass_rust
import concourse.bass as bass
import concourse.tile as tile
from concourse import bacc
from concourse import mybir
from concourse.bass_utils import run_bass_kernel_spmd

F32 = mybir.dt.float32
BF16 = mybir.dt.bfloat16
ALU = mybir.AluOpType
ACTF = mybir.ActivationFunctionType
AX = mybir.AxisListType

EPS = 1e-8
THETA = 0.5 + 1e-8

N = 2
DVOL = 192
W = 192
DOUT = 190          # valid conv output extent per axis
D_IN = 50           # input slab depth rows per core
D_OUT_CORE = 48     # output depth rows computed per core
DB = 6              # output d rows per subblock
NSUB = D_OUT_CORE // DB
FD = DB * W         # pointwise free-dim extent
ROW = 3 * W         # one interleaved d-row in X3: shifts s=0,1,2 concatenated
X3W = (DB + 2) * ROW  # data cols in X3
X3PAD = X3W + 2     # +2 pad cols so trailing w+2 reads stay in-bounds
U3E = DB * ROW + 2  # U extent incl. w+1 read at s=2
DB2 = DB + 2        # sign path needs DB+2 d-rows
NPACK = NSUB // 2   # packed iterations covering h rows 126..189
NCOL = D_OUT_CORE + NPACK * DB  # accP/accC column count (48 + 24)

# per-core input-slab depth starts; output rows covered = d0..d0+47
CORE_D0 = [0, 48, 96, 142]

_last_results = None  # test harness reads exec time from here


def xo(s, d, w):
    return d * ROW + s * W + w


def _emit(tc, x, band, outp, outc, dbg=None):
    nc = tc.nc
    import contextlib
    import math

    with contextlib.ExitStack() as ctx:
        xpool = ctx.enter_context(tc.tile_pool(name="xin", bufs=2))
        cpool = ctx.enter_context(tc.tile_pool(name="xc", bufs=2))
        mpool = ctx.enter_context(tc.tile_pool(name="main", bufs=2))
        apool = ctx.enter_context(tc.tile_pool(name="acc", bufs=1))
        ppool = ctx.enter_context(tc.tile_pool(name="ps", bufs=2, space="PSUM"))

        # merged accumulator: per iteration, cols [2c, 2c+6) hold mask counts
        # and [2c+6, 2c+12) hold pen sums (one fused tensor_reduce per iter)
        acc = apool.tile([128, 2 * NCOL], F32, tag="acc", name="acc")
        nc.vector.memset(acc[:], 0.0)
        # band load via the gpsimd queue: keeps the sync DMA queue free for
        # the first Xb/Xc input transfers (band isn't needed until the first
        # matmul, ~15us in)
        bandt = apool.tile([128, 4 * 128], BF16, tag="band", name="bandt")
        nc.gpsimd.dma_start(bandt[:, :], band)

        # bias constants for ACT (only 0.0/1.0 are pre-registered)
        bias_tiles = {}
        for i, bval in enumerate(
            (2e-4, EPS, -1.5 * math.log(2.0), math.log(8.0 / THETA**2))
        ):
            bt = apool.tile([128, 1], F32, tag=f"bias{i}", name=f"bias{i}")
            nc.gpsimd.memset(bt[:], bval)
            bias_tiles[bval] = bt

        def BIAS(v):
            return bias_tiles[v][:, :]

        def T(tag, fd=FD, dt=BF16):
            return mpool.tile([128, fd], dt, tag=tag, name=tag)

        TT = nc.vector.tensor_tensor
        TS = nc.vector.tensor_scalar
        TSS = nc.vector.tensor_single_scalar
        ACT = nc.scalar.activation

        # iteration schedule: (packed, h0, din_a, din_b, col). Packed
        # iterations carry ~2.5x the PE work of unpacked ones, so interleave
        # them among the unpacked iterations to absorb the PE debt.
        unpacked = [(False, 0, DB * j, 0, DB * j) for j in range(NSUB)]
        packed_l = [
            (True, 126, 2 * DB * jj, 2 * DB * jj + DB, D_OUT_CORE + DB * jj)
            for jj in range(NPACK)
        ]
        iters = []
        for k in range(NSUB):
            iters.append(unpacked[k])
            if k % 2 == 0 and packed_l:
                iters.append(packed_l.pop(0))
        iters.extend(packed_l)

        for it, (packed, h0, din_a, din_b, col) in enumerate(iters):
            def DUMP(nm, t):
                if dbg is not None and it == 0 and nm in dbg:
                    nc.gpsimd.dma_start(dbg[nm], t)

            Xb = xpool.tile([128, X3PAD], BF16, tag="Xb", name="Xb")
            nc.gpsimd.memset(Xb[:, X3W:X3PAD], 1.0)
            if packed:
                for half, din0 in ((0, din_a), (1, din_b)):
                    src = x.copy()
                    src.offset = din0 * DVOL * W + h0 * W
                    src.ap = bass_rust.VecI64Pair(
                        [[W, 64], [DVOL * W, DB + 2], [1, ROW]]
                    )
                    nc.sync.dma_start(
                        Xb[64 * half : 64 * half + 64, 0:X3W], src
                    )
            else:
                src = x.copy()
                src.offset = din_a * DVOL * W + h0 * W
                src.ap = bass_rust.VecI64Pair(
                    [[W, 128], [DVOL * W, DB + 2], [1, ROW]]
                )
                nc.sync.dma_start(Xb[0:128, 0:X3W], src)
            # w+1-shifted replica loaded straight from DRAM (same linear
            # stream offset by one element; host pads the slab by 2) so it
            # runs in parallel with the Xb DMA instead of serializing on it
            Xc = cpool.tile([128, X3PAD], BF16, tag="Xc", name="Xc")
            nc.gpsimd.memset(Xc[:, X3W:X3PAD], 1.0)
            if packed:
                for half, din0 in ((0, din_a), (1, din_b)):
                    src = x.copy()
                    src.offset = din0 * DVOL * W + h0 * W + 1
                    src.ap = bass_rust.VecI64Pair(
                        [[W, 64], [DVOL * W, DB + 2], [1, ROW]]
                    )
                    nc.sync.dma_start(
                        Xc[64 * half : 64 * half + 64, 0:X3W], src
                    )
            else:
                src = x.copy()
                src.offset = din_a * DVOL * W + h0 * W + 1
                src.ap = bass_rust.VecI64Pair(
                    [[W, 128], [DVOL * W, DB + 2], [1, ROW]]
                )
                nc.sync.dma_start(Xc[0:128, 0:X3W], src)

            def xb(s, d, w, n=W):
                return _view2(Xb, xo(s, d, w), ROW, DB, n)

            def xc(s, d, w, n=W):
                return _view2(Xc, xo(s, d, w), ROW, DB, n)

            # ---- sign field early (feeds PE) ----
            # full X3 width: the packed-path fix matmuls need s=1,2 replicas
            sgf = T("sgF", X3PAD)
            ACT(sgf[:, :], Xb[:, :], ACTF.Sign)

            # ---- stencil fields (bf16, all reads 4B-aligned) ----
            U3 = T("U3", U3E)  # d-derivative of Xc, all 3 shifts
            TT(U3[:, 0:U3E], Xc[:, 2 * ROW : 2 * ROW + U3E],
               Xc[:, 0:U3E], ALU.subtract)

            def uoc(s, d, w, n=W):
                return _view2(U3, xo(s, d, w), ROW, DB, n)

            Vr = T("Vr", DB * 194)  # 2gy on 194-wide rows (w0 base)
            TT(_view2(Vr, 0, 194, DB, 194),
               _view2(Xb, xo(2, 1, 0), ROW, DB, 194),
               _view2(Xb, xo(0, 1, 0), ROW, DB, 194), ALU.subtract)

            def vv(w, n=W):
                return _view2(Vr, w, 194, DB, n)

            Vc = T("Vc")  # 2gy centered (aligned)
            TT(dnv(Vc), xc(2, 1, 0), xc(0, 1, 0), ALU.subtract)

            t1 = T("t1")
            TT(dnv(t1), xc(1, 0, 0), xc(1, 2, 0), ALU.add)
            t2 = T("t2")
            TT(dnv(t2), xb(1, 1, 0), xb(1, 1, 2), ALU.add)
            t3 = T("t3")
            TT(dnv(t3), xc(0, 1, 0), xc(2, 1, 0), ALU.add)
            x2c = T("s4")  # 2*x(d+1,h+1,w+1)  (on ACT: frees DVE cycles)
            ACT(dnv(x2c), xc(1, 1, 0), ACTF.Copy, scale=2.0)
            A = T("A")  # hxx
            TT(A[:, :], t1[:, :], x2c[:, :], ALU.subtract)
            C0 = T("C0")  # hzz
            TT(C0[:, :], t2[:, :], x2c[:, :], ALU.subtract)
            B = T("B")  # hyy
            TT(B[:, :], t3[:, :], x2c[:, :], ALU.subtract)
            W1 = T("W1")  # 2gz
            TT(dnv(W1), xb(1, 1, 2), xb(1, 1, 0), ALU.subtract)
            P = T("P")  # 4hxy (aligned via U3c)
            TT(dnv(P), uoc(2, 0, 0), uoc(0, 0, 0), ALU.subtract)
            # 4hxz = u[w+2]-u[w]: odd-offset views of U3 (1x DVE mode but
            # one op instead of three)
            Q = T("Q")  # 4hxz
            TT(Q[:, :],
               _view2(U3, xo(1, 0, 0) + 1, ROW, DB, W),
               _view2(U3, xo(1, 0, 0) - 1, ROW, DB, W), ALU.subtract)
            R = T("R")  # 4hyz
            TT(dnv(R), vv(2), vv(0), ALU.subtract)

            # ---- squares scaled by 2 (ACT, one table): U2 = 2u^2 etc., so
            # G = 2*(S2*trH - F) falls out of the X-products with no 0.5 op;
            # downstream constants absorb the factor 2 ----
            SQ2 = math.sqrt(2.0)
            U2 = T("U2")
            ACT(dnv(U2), uoc(1, 0, 0), ACTF.Square, scale=SQ2)
            V2 = T("V2")
            ACT(V2[:, :], Vc[:, :], ACTF.Square, scale=SQ2)
            W2s = T("W2s")
            ACT(W2s[:, :], W1[:, :], ACTF.Square, scale=SQ2)

            # ---- S2 = 8|g|^2; scaled-curvature reciprocal cluster ----
            S2 = T("S2")
            TT(S2[:, :], U2[:, :], V2[:, :], ALU.add)
            TT(S2[:, :], S2[:, :], W2s[:, :], ALU.add)
            DUMP("S2", S2[:, :])
            # the 2e-4 bias keeps mag^3 >> EPS so the scaled algebra stays
            # exact; only near-zero-gradient voxels (a handful, ref pen ~1e5
            # of a ~3e9 total) are perturbed
            Ltile = T("cC", FD, F32)  # ln(2*S2old+2e-4)
            ACT(Ltile[:, :], S2[:, :], ACTF.Ln, bias=BIAS(2e-4))
            m3h = T("cA")  # 2*M3 = (S2old')^1.5 = exp(1.5L)/2^1.5
            ACT(m3h[:, :], Ltile[:, :], ACTF.Exp, scale=1.5,
                bias=BIAS(-1.5 * math.log(2.0)))
            rr = T("R3")  # 1/(theta*2*M3)^2 = (8/theta^2)*exp(-3L)
            ACT(rr[:, :], Ltile[:, :], ACTF.Exp, scale=-3.0,
                bias=BIAS(math.log(8.0 / THETA**2)))

            # Fc = uvP + uwQ + vwR = u*(vP + wQ) + (vw)*R
            vP = T("s0")
            TT(vP[:, :], Vc[:, :], P[:, :], ALU.mult)
            wQ = T("s1")
            TT(wQ[:, :], W1[:, :], Q[:, :], ALU.mult)
            TT(vP[:, :], vP[:, :], wQ[:, :], ALU.add)
            Fc = T("s2")
            TT(dnv(Fc), uoc(1, 0, 0), _view2(vP, 0, W, DB, W), ALU.mult)
            vw = T("s1")
            TT(vw[:, :], Vc[:, :], W1[:, :], ALU.mult)
            TT(vw[:, :], vw[:, :], R[:, :], ALU.mult)
            TT(Fc[:, :], Fc[:, :], vw[:, :], ALU.add)

            # G = 2*(S2*trH - F) = U2*(B+C) + V2*(A+C) + W2s*(A+B) - Fc
            BC = T("trH")
            TT(BC[:, :], B[:, :], C0[:, :], ALU.add)
            AC = T("t2")
            TT(AC[:, :], A[:, :], C0[:, :], ALU.add)
            AB = T("t3")
            TT(AB[:, :], A[:, :], B[:, :], ALU.add)
            G = T("s0")
            TT(G[:, :], U2[:, :], BC[:, :], ALU.mult)
            X2 = T("s1")
            TT(X2[:, :], V2[:, :], AC[:, :], ALU.mult)
            TT(G[:, :], G[:, :], X2[:, :], ALU.add)
            TT(X2[:, :], W2s[:, :], AB[:, :], ALU.mult)
            TT(G[:, :], G[:, :], X2[:, :], ALU.add)
            TT(G[:, :], G[:, :], Fc[:, :], ALU.subtract)

            # ---- curvature glue (bf16) ----
            Gsq = T("s2")
            ACT(Gsq[:, :], G[:, :], ACTF.Square)
            tm = T("s3")
            TT(tm[:, :], m3h[:, :], G[:, :], ALU.mult)  # (2M3)*(2G)
            Dq = T("s1")
            TT(Dq[:, :], Gsq[:, :], tm[:, :], ALU.subtract)  # 4*dq*M3^2
            ad = T("s3")
            ACT(ad[:, :], Dq[:, :], ACTF.Abs)
            lnD = T("cL", FD, F32)
            ACT(lnD[:, :], ad[:, :], ACTF.Ln, bias=BIAS(EPS))
            sqv = T("s1")
            ACT(sqv[:, :], lnD[:, :], ACTF.Exp, scale=0.5)
            num = T("s0")
            TT(num[:, :], G[:, :], sqv[:, :], ALU.add)  # k1*M3
            numsq = T("s2")
            ACT(numsq[:, :], num[:, :], ACTF.Square)
            k2 = T("s1")
            TT(k2[:, :], numsq[:, :], rr[:, :], ALU.mult)
            pen = T("s0")
            TS(pen[:, :], k2[:, :], -1.0, 0.0, ALU.add, ALU.max)
            DUMP("pen", pen[:, :])

            # ---- zero-crossing mask: 27-sum of signs via PE matmuls ----
            # h-window via 3-diag band over partitions; (d,w)-window via 9
            # shifted views. Packed iterations use the clipped band plus two
            # fix matmuls routing partition-63/127 s-replicas to the half's
            # top rows.
            sdp = ppool.tile([128, DB * 256], F32, tag="sdps", name="sdp")
            if packed:
                bmain = bandt[:, 128:256]
                b12 = bandt[:, 256:384]
                b3 = bandt[:, 384:512]
                for dd in range(3):
                    for dw in range(3):
                        last = dd == 2 and dw == 2
                        for dp in range(0, DB, 2):
                            nc.tensor.matmul(
                                _view2(sdp, dp * 256, 256, 2, W),
                                bmain,
                                _view2(sgf, xo(0, dd + dp, dw), ROW, 2, W),
                                start=(dd == 0 and dw == 0),
                                stop=False,
                            )
                        for dp in range(0, DB, 2):
                            nc.tensor.matmul(
                                _view2(sdp, dp * 256, 256, 2, W),
                                b12,
                                _view2(sgf, xo(1, dd + dp, dw), ROW, 2, W),
                                start=False,
                                stop=False,
                            )
                            nc.tensor.matmul(
                                _view2(sdp, dp * 256, 256, 2, W),
                                b3,
                                _view2(sgf, xo(2, dd + dp, dw), ROW, 2, W),
                                start=False,
                                stop=last,
                            )
            else:
                for dd in range(3):
                    for dw in range(3):
                        for dp in range(0, DB, 2):
                            nc.tensor.matmul(
                                _view2(sdp, dp * 256, 256, 2, W),
                                bandt[:, 0:128],
                                _view2(sgf, xo(0, dd + dp, dw), ROW, 2, W),
                                start=(dd == 0 and dw == 0),
                                stop=(dd == 2 and dw == 2),
                            )
            sd2 = T("t1")
            ACT(dnv(sd2), _view2(sdp, 0, 256, DB, W), ACTF.Square)
            # mask in rows 0..5 of mp, pen*mask in rows 6..11: one fused
            # per-d-row reduction covers both
            mp = T("mp", 2 * FD)
            mask = mp[:, 0:FD]
            TSS(mask, sd2[:, :], 728.5, ALU.is_lt)
            DUMP("mask", mask)
            TT(mp[:, FD : 2 * FD], pen[:, :], mask, ALU.mult)
            nc.vector.tensor_reduce(
                acc[:, 2 * col : 2 * col + 2 * DB],
                _view2(mp, 0, W, 2 * DB, DOUT), AX.X, ALU.add,
            )

        nc.sync.dma_start(outp, acc[:, :])


def dnv(t, w=0, n=W):
    """dense [d][192] tile view"""
    return _view2(t, w, W, DB, n)


def _install_ntff_hook_shim():
    """Recreate antenv.axon_hooks (absent in this image) so trace=True works."""
    import sys as _sys
    import types
    if "antenv.axon_hooks" in _sys.modules:
        return
    try:
        from trn_agent_boot.trn_boot import _ntff_profile_via_ctypes
        hook = _ntff_profile_via_ctypes("/opt/axon/libaxon_pjrt.so")
    except Exception as e:
        print("ntff shim failed:", e)
        hook = None
    mod = types.ModuleType("antenv.axon_hooks")
    _state = {"hook": hook}
    mod.get_axon_ntff_profile_hook = lambda: _state["hook"]
    mod.set_axon_ntff_profile_hook = lambda h: _state.update(hook=h)
    _sys.modules["antenv.axon_hooks"] = mod
    import antenv
    antenv.axon_hooks = mod


def _view2(t, off, dstep, dcnt, n):
    """AP view of tile t: all partitions, free dims [(dstep, dcnt), (1, n)] at off."""
    ap = t[:, 0:1].copy()
    base = ap.ap.to_list()
    pdim = base[0]
    ap.offset = ap.offset + off
    ap.ap = bass_rust.VecI64Pair([list(pdim), [dstep, dcnt], [1, n]])
    return ap


def _patch_act_tables():
    """Steer the act-table-load pass away from the exp-less `natural_log`
    set: with ln hidden from it, an ln-miss resolves to
    `natural_log_exp_and_others`, which covers every function this kernel
    uses (ln/exp/square/sign/abs) -> ~2 table loads total instead of 4 per
    iteration. Entry positions (= act_func_set ids) are unchanged, so the
    emitted ids stay consistent with the compiler's act_info.json.
    """
    if getattr(bacc, "_act_tables_patched", False):
        return
    orig = bacc.get_activation_tables

    def patched(arch):
        out = {}
        for name, s in orig(arch).items():
            if name == "natural_log":
                s = {f for f in s if f != ACTF.Ln}
            out[name] = s
        return out

    bacc.get_activation_tables = patched
    bacc._act_tables_patched = True


def _build_nc():
    _patch_act_tables()
    nc = bacc.Bacc("TRN2", target_bir_lowering=False, debug=False, num_devices=8)
    x = nc.dram_tensor("x", [D_IN * DVOL * W + 2], BF16, kind="ExternalInput")
    band = nc.dram_tensor("band", [128, 4 * 128], BF16, kind="ExternalInput")
    outp = nc.dram_tensor("outp", [128, 2 * NCOL], F32, kind="ExternalOutput")
    with tile.TileContext(nc) as tc:
        _emit(tc, x.ap(), band.ap(), outp.ap(), None)
    nc.finalize()
    return nc


def _band_matrices():
    import ml_dtypes
    bm = np.zeros((128, 4 * 128), dtype=ml_dtypes.bfloat16)
    # [0:128]   unclipped 3-diag: out[o] = sum_{k=o..o+2} in[k]
    # [128:256] clipped at the 64-partition half boundary
    # [256:384] b12: routes partition 63/127 (s=1 view) to outputs 62,63
    # [384:512] b3: routes partition 63/127 (s=2 view) to output 63
    for o in range(128):
        for k in range(o, min(o + 3, 128)):
            bm[k, o] = 1.0
            if k // 64 == o // 64:
                bm[k, 128 + o] = 1.0
    for base in (0, 64):
        bm[base + 63, 256 + base + 62] = 1.0
        bm[base + 63, 256 + base + 63] = 1.0
        bm[base + 63, 384 + base + 63] = 1.0
    return bm


def kernel(phi):
    global _last_results
    phi = np.asarray(phi)
    assert phi.shape == (N, 1, DVOL, DVOL, W), phi.shape
    nc = _build_nc()
    import ml_dtypes
    phib = phi.astype(ml_dtypes.bfloat16)
    bandm = _band_matrices()
    in_maps = []
    for c in range(8):
        n, q = divmod(c, 4)
        d0 = CORE_D0[q]
        # flat + 2-element pad: the w+1-shifted Xc DMA reads one element
        # past the slab end on the last packed iteration
        slab = np.concatenate(
            [phib[n, 0, d0 : d0 + D_IN].ravel(),
             np.zeros(2, dtype=phib.dtype)]
        )
        in_maps.append({"x": slab, "band": bandm})
    trace = bool(int(os.environ.get("KERNEL_TRACE", "0")))
    if trace:
        _install_ntff_hook_shim()
    res = run_bass_kernel_spmd(nc, in_maps, list(range(8)), trace=trace)
    _last_results = res
    tp = 0.0
    tcnt = 0.0
    for c in range(8):
        blk = res.results[c]["outp"].astype(np.float64)
        blk = blk.reshape(128, NSUB + NPACK, 2, DB)
        oc = blk[:, :, 0, :]  # mask counts
        op = blk[:, :, 1, :]  # pen sums
        dlo = 2 if (c % 4) == 3 else 0
        # h-block 0 (iters 0..7): iter j holds d = 6j+ci, h rows 0..125 valid
        ocu = oc[:, :NSUB].reshape(128, D_OUT_CORE)
        opu = op[:, :NSUB].reshape(128, D_OUT_CORE)
        tp += opu[:126, dlo:].sum()
        tcnt += ocu[:126, dlo:].sum()
        # packed h-block (h 126..189, iters 8..11): iter 8+jj holds
        # d = 12jj+ci for partitions 0..63 and d = 12jj+6+ci for 64..127
        opp = op[:, NSUB:]
        ocp = oc[:, NSUB:]
        tp += opp[64:].sum() + opp[:64, 1:].sum() + opp[:64, 0, dlo:].sum()
        tcnt += ocp[64:].sum() + ocp[:64, 1:].sum() + ocp[:64, 0, dlo:].sum()
    return np.float32(tp / (tcnt + EPS))
